# revision 5
# baseline (speedup 1.0000x reference)
"""Trainium2 Bass kernel for nn_CausalSelfAttention_2783138808334.

B=8, T=1024, C=64, n_head=1. Data-parallel over batch: one batch per
NeuronCore across 8 cores (weights/tables replicated), gathered on the host.

See emit() for the per-core algorithm.
"""
import numpy as np

import concourse.bass as bass
import concourse.bacc as bacc
import concourse.mybir as mybir
from concourse import masks
from concourse.ap import AP

F32 = mybir.dt.float32
BF = mybir.dt.bfloat16
T = 1024
C = 64
NT = 8          # 128-row tiles of T
D = 2048        # scratch DRAM row pitch (elements)
SCALE = 0.125   # 1/sqrt(C)
FILL = -4000.0  # pre-scale mask fill: exp(0.125 * -4000) == 0 in fp32


def rev_free(ap):
    """Reverse the (contiguous) free dim of a 2D AP."""
    (ps, pc), (fs, fc) = ap.ap
    assert fs == 1, ap.ap
    return AP(ap.tensor, ap.offset + (fc - 1), [[ps, pc], [-1, fc]])


def mm_chunks(lo, hi, step=512):
    """Split [lo, hi) at 512-element PSUM bank boundaries."""
    a = lo
    while a < hi:
        b = min(hi, (a // step + 1) * step)
        yield a, b
        a = b


def emit(nc, tc, xd, wqkv, bqkv, embk, embv, wproj, bproj, yd):
    with (
        tc.tile_pool(name="const", bufs=1) as cp,
        tc.tile_pool(name="work", bufs=3) as wp,
        tc.tile_pool(name="psum", bufs=1, space="PSUM") as pp,
        tc.tile_pool(name="dram", bufs=1, space="DRAM") as dp,
    ):
        QED = dp.tile([T, D], BF, name="QED").tensor
        A1D = dp.tile([T, D], BF, name="A1D").tensor

        ident = cp.tile([128, 128], F32)
        masks.make_identity(nc, ident)
        ones_col = cp.tile([128, 1], BF)
        nc.gpsimd.memset(ones_col, 1.0)
        ones_row = cp.tile([1, T], BF)
        nc.gpsimd.memset(ones_row, 1.0)

        # ---- loads (fp32) ----
        X = cp.tile([128, 512], F32)    # x[128n+p, c] at [p, 64n+c]
        EK = cp.tile([128, 512], F32)
        EV = cp.tile([128, 512], F32)
        nc.sync.dma_start(out=X.rearrange("p (n c) -> p n c", c=C),
                          in_=xd.rearrange("(n p) c -> p n c", p=128))
        nc.sync.dma_start(out=EK.rearrange("p (n c) -> p n c", c=C),
                          in_=embk.rearrange("(n p) c -> p n c", p=128))
        nc.sync.dma_start(out=EV.rearrange("p (n c) -> p n c", c=C),
                          in_=embv.rearrange("(n p) c -> p n c", p=128))
        W0 = cp.tile([128, C], F32)
        W1 = cp.tile([C, C], F32)
        WP = cp.tile([C, C], F32)
        nc.sync.dma_start(out=W0[:, :], in_=wqkv[0:128, :])
        nc.sync.dma_start(out=W1[:, :], in_=wqkv[128:192, :])
        nc.sync.dma_start(out=WP[:, :], in_=wproj[:, :])
        bq = cp.tile([1, 3 * C], F32)
        bp = cp.tile([1, C], F32)
        nc.sync.dma_start(out=bq[:, :], in_=bqkv.unsqueeze(0))
        nc.sync.dma_start(out=bp[:, :], in_=bproj.unsqueeze(0))

        # ---- on-chip transposes + bf16 casts ----
        xT = cp.tile([C, T], BF)
        ekT = cp.tile([C, T], BF)
        for n in range(NT):
            ps = pp.tile([C, 128], F32, tag="small", bufs=2)
            nc.tensor.transpose(ps[:, :], X[:, 64 * n:64 * n + 64], ident[:, :])
            nc.scalar.copy(xT[:, 128 * n:128 * (n + 1)], ps[:, :])
            ps2 = pp.tile([C, 128], F32, tag="small", bufs=2)
            nc.tensor.transpose(ps2[:, :], EK[:, 64 * n:64 * n + 64], ident[:, :])
            nc.scalar.copy(ekT[:, 128 * n:128 * (n + 1)], ps2[:, :])
        WT = cp.tile([C, 3 * C], BF)
        ps = pp.tile([C, 128], F32, tag="small", bufs=2)
        nc.tensor.transpose(ps[:, :], W0[:, :], ident[:, :])
        nc.scalar.copy(WT[:, 0:128], ps[:, :])
        ps = pp.tile([C, 128], F32, tag="small", bufs=2)
        nc.tensor.transpose(ps[:, 0:C], W1[:, :], ident[0:C, 0:C])
        nc.scalar.copy(WT[:, 128:192], ps[:, 0:C])
        WpT = cp.tile([C, C], F32)
        ps = pp.tile([C, 128], F32, tag="small", bufs=2)
        nc.tensor.transpose(ps[:, 0:C], WP[:, :], ident[0:C, 0:C])
        nc.vector.tensor_copy(WpT[:, :], ps[:, 0:C])
        EMBV = cp.tile([128, 512], BF)
        nc.vector.tensor_copy(EMBV[:, :], EV[:, :])
        bqb = cp.tile([1, 3 * C], BF)
        nc.vector.tensor_copy(bqb[:, :], bq[:, :])

        # ---- qkv projection ----
        qT = cp.tile([C, T], BF)
        kT = cp.tile([C, T], BF)
        ps_qk = pp.tile([128, T], F32, tag="big", bufs=2)
        for a, b in mm_chunks(0, T):
            nc.tensor.matmul(ps_qk[:, a:b], WT[:, 0:128], xT[:, a:b], start=True, stop=False)
            nc.tensor.matmul(ps_qk[:, a:b], bqb[:, 0:128], ones_row[:, a:b], start=False, stop=True)
        nc.scalar.copy(qT[:, :], ps_qk[0:C, :])
        nc.scalar.copy(kT[:, :], ps_qk[C:128, :])
        V = cp.tile([128, 512], BF)     # v[128n+p, c] at [p, 64n+c]
        for n in range(NT):
            ps_v = pp.tile([128, C], F32, tag="small", bufs=2)
            nc.tensor.matmul(ps_v[:, :], xT[:, 128 * n:128 * (n + 1)], WT[:, 128:192],
                             start=True, stop=False)
            nc.tensor.matmul(ps_v[:, :], ones_row[:, 0:128], bqb[:, 128:192],
                             start=False, stop=True)
            nc.scalar.copy(V[:, 64 * n:64 * (n + 1)], ps_v[:, :])

        # ---- fused score/softmax/value loop ----
        # Iteration i does: (a) normal-orientation QE/att1 for t-tile i,
        # reversed-cast, DRAM write (SWDGE); (b) transposed scores for
        # s/u-tile k=i: skewed transpose-DMA readback, mask, exp; (c) Z and
        # value-matmul accumulation for tile k=i. Emitted i=7..0 so each
        # s-tile's readback (rows >= 128k) only waits on already-issued writes.
        ET = [cp.tile([128, T], BF, tag=f"et{k}", name=f"et{k}") for k in range(NT)]
        EUT = [cp.tile([128, T], BF, tag=f"eut{k}", name=f"eut{k}") for k in range(NT)]
        for k in range(NT):
            if k % 4 != 0:
                g0 = 512 * (k // 4)
                nc.gpsimd.memset(ET[k][:, g0:128 * k], 0.0)
                nc.gpsimd.memset(EUT[k][:, g0:128 * k], 0.0)
        ps_z = pp.tile([1, T], F32, tag="zrow", bufs=1)
        ps_y = [pp.tile([C, 512], F32, tag="small", bufs=2, name=f"ps_y{g}")
                for g in range(2)]
        for i in range(NT - 1, -1, -1):
            # --- (a) t-tile i: QE[t, d], att1[t, s] -> reversed bf16 -> DRAM
            Wd = 128 * (i + 1)          # triangular: only d,s <= t needed
            ps_qe = pp.tile([128, T], F32, tag="big", bufs=2)
            ps_a1 = pp.tile([128, T], F32, tag="big", bufs=2)
            for a, b in mm_chunks(0, Wd):
                nc.tensor.matmul(ps_qe[:, a:b], qT[:, 128 * i:128 * (i + 1)],
                                 ekT[:, a:b], start=True, stop=True)
                nc.tensor.matmul(ps_a1[:, a:b], qT[:, 128 * i:128 * (i + 1)],
                                 kT[:, a:b], start=True, stop=True)
            qer = wp.tile([128, T], BF, tag="qer")
            a1r = wp.tile([128, T], BF, tag="a1r")
            nc.vector.tensor_copy(qer[:, 0:Wd], rev_free(ps_qe[:, 0:Wd]))
            nc.vector.tensor_copy(a1r[:, 0:Wd], rev_free(ps_a1[:, 0:Wd]))
            nc.gpsimd.dma_start(out=AP(QED, 128 * i * D + (D - Wd), [[D, 128], [1, Wd]]),
                                in_=qer[:, 0:Wd])
            nc.gpsimd.dma_start(out=AP(A1D, 128 * i * D + (D - Wd), [[D, 128], [1, Wd]]),
                                in_=a1r[:, 0:Wd])

            # --- (b) s/u-tile k=i: transposed scores + skewed readback
            k = i
            k0 = 128 * k
            Wt = T - k0                 # valid t range [k0, T)
            ps_s = pp.tile([128, T], F32, tag="big", bufs=2)
            ps_u = pp.tile([128, T], F32, tag="big", bufs=2)
            for a, b in mm_chunks(k0, T):
                nc.tensor.matmul(ps_s[:, a:b], kT[:, k0:k0 + 128], qT[:, a:b],
                                 start=True, stop=True)
                nc.tensor.matmul(ps_u[:, a:b], ekT[:, k0:k0 + 128], qT[:, a:b],
                                 start=True, stop=True)
            a2t = wp.tile([128, T], BF, tag="a2t")
            a1ut = wp.tile([128, T], BF, tag="a1ut")
            nc.sync.dma_start(out=a2t[:, k0:T],
                              in_=AP(QED, k0 * (D - 1) + (D - 1) + k0, [[D - 1, Wt], [1, 128]]),
                              transpose=True)
            nc.sync.dma_start(out=a1ut[:, k0:T],
                              in_=AP(A1D, k0 * (D - 1) + (D - 1) + k0, [[D - 1, Wt], [1, 128]]),
                              transpose=True)
            tt = wp.tile([128, T], F32, tag="tt")
            su = wp.tile([128, T], F32, tag="su")
            nc.vector.tensor_add(tt[:, k0:T], ps_s[:, k0:T], a2t[:, k0:T])
            nc.vector.tensor_add(su[:, k0:T], ps_u[:, k0:T], a1ut[:, k0:T])
            # keep where t - s >= 0 (in-slice iota: t'' - p)
            nc.gpsimd.affine_select(out=tt[:, k0:T], in_=tt[:, k0:T], pattern=[[1, Wt]],
                                    compare_op=mybir.AluOpType.is_ge, fill=FILL,
                                    base=0, channel_multiplier=-1)
            nc.gpsimd.affine_select(out=su[:, k0:T], in_=su[:, k0:T], pattern=[[1, Wt]],
                                    compare_op=mybir.AluOpType.is_ge, fill=FILL,
                                    base=0, channel_multiplier=-1)
            nc.scalar.activation(ET[k][:, k0:T], tt[:, k0:T],
                                 mybir.ActivationFunctionType.Exp, scale=SCALE)
            nc.scalar.activation(EUT[k][:, k0:T], su[:, k0:T],
                                 mybir.ActivationFunctionType.Exp, scale=SCALE)
            # --- (c) Z and value accumulation for tile k
            # PSUM accumulation groups are bank-granular: k = 4g+3 opens its
            # whole 512-col bank (the zeroed strip makes that exact).
            if k % 4 == 3:
                g0 = 512 * (k // 4)
                nc.tensor.matmul(ps_z[:, g0:g0 + 512], ones_col[:, :],
                                 ET[k][:, g0:g0 + 512], start=True, stop=False)
                for a, b in mm_chunks(g0 + 512, T):
                    nc.tensor.matmul(ps_z[:, a:b], ones_col[:, :], ET[k][:, a:b],
                                     start=False, stop=False)
            else:
                for a, b in mm_chunks(k0, T):
                    nc.tensor.matmul(ps_z[:, a:b], ones_col[:, :], ET[k][:, a:b],
                                     start=False, stop=(k == 0))
            for g in range(k // 4, 2):
                gs = slice(512 * g, 512 * (g + 1))
                first = (k == 4 * g + 3)
                nc.tensor.matmul(ps_y[g][:, :], V[:, 64 * k:64 * (k + 1)], ET[k][:, gs],
                                 start=first, stop=False)
                nc.tensor.matmul(ps_y[g][:, :], EMBV[:, 64 * k:64 * (k + 1)], EUT[k][:, gs],
                                 start=False, stop=(k == 0))

        ysT = cp.tile([C, T], F32)
        for g in range(2):
            nc.scalar.copy(ysT[:, 512 * g:512 * (g + 1)], ps_y[g][:, :])

        # ---- 1/Z per t-tile (row -> column via K=1 matmul with ones[1,1]) ----
        Zrow = cp.tile([1, T], F32)
        nc.vector.tensor_copy(Zrow[:, :], ps_z[:, :])
        one11 = cp.tile([1, 1], F32)
        nc.gpsimd.memset(one11, 1.0)
        zc = cp.tile([128, NT], F32)
        for i in range(NT):
            ps_zt = pp.tile([128, 1], F32, tag="small", bufs=2)
            nc.tensor.matmul(ps_zt[:, :], Zrow[:, 128 * i:128 * (i + 1)], one11[:, :],
                             start=True, stop=True)
            nc.vector.tensor_copy(zc[:, i:i + 1], ps_zt[:, :])
        rz = cp.tile([128, NT], F32)
        nc.vector.reciprocal(rz[:, :], zc[:, :])

        # ---- output projection + bias + 1/Z ----
        for i in range(NT):
            ps_p = pp.tile([128, C], F32, tag="small", bufs=2)
            nc.tensor.matmul(ps_p[:, :], ysT[:, 128 * i:128 * (i + 1)], WpT[:, :],
                             start=True, stop=False)
            # + Z[t] * bproj[j]  (so the 1/Z below leaves bias intact)
            nc.tensor.matmul(ps_p[:, :], Zrow[:, 128 * i:128 * (i + 1)], bp[:, :],
                             start=False, stop=True)
            yt = wp.tile([128, C], F32, tag="yt")
            nc.vector.tensor_scalar_mul(yt[:, :], ps_p[:, :], rz[:, i:i + 1])
            nc.sync.dma_start(out=yd[128 * i:128 * (i + 1), :], in_=yt[:, :])


_NC_CACHE = None


def _build():
    global _NC_CACHE
    if _NC_CACHE is not None:
        return _NC_CACHE
    nc = bacc.Bacc("TRN2", target_bir_lowering=False, debug=False)
    xd = nc.dram_tensor("x", [T, C], F32, kind="ExternalInput")
    wqkv = nc.dram_tensor("Wqkv", [3 * C, C], F32, kind="ExternalInput")
    bqkv = nc.dram_tensor("bqkv", [3 * C], F32, kind="ExternalInput")
    embk = nc.dram_tensor("embk", [T, C], F32, kind="ExternalInput")
    embv = nc.dram_tensor("embv", [T, C], F32, kind="ExternalInput")
    wproj = nc.dram_tensor("Wproj", [C, C], F32, kind="ExternalInput")
    bproj = nc.dram_tensor("bproj", [C], F32, kind="ExternalInput")
    yd = nc.dram_tensor("y", [T, C], F32, kind="ExternalOutput")
    from concourse.tile import TileContext
    with TileContext(nc) as tc:
        emit(nc, tc, xd.ap(), wqkv.ap(), bqkv.ap(), embk.ap(), embv.ap(),
             wproj.ap(), bproj.ap(), yd.ap())
    nc.compile()
    _NC_CACHE = nc
    return nc


def run_spmd(inputs, **kwargs):
    from concourse.bass_utils import run_bass_kernel_spmd
    x = np.asarray(inputs["x"], dtype=np.float32)
    B = x.shape[0]
    nc = _build()
    shared = {k: np.ascontiguousarray(np.asarray(inputs[k], dtype=np.float32))
              for k in ("Wqkv", "bqkv", "embk", "embv", "Wproj", "bproj")}
    in_maps = [dict(shared, x=np.ascontiguousarray(x[b])) for b in range(B)]
    res = run_bass_kernel_spmd(nc, in_maps, core_ids=list(range(B)), **kwargs)
    y = np.stack([r["y"] for r in res.results], axis=0)
    return y, res


def kernel(**inputs):
    y, _ = run_spmd(inputs)
    return y
